# revision 101
# baseline (speedup 1.0000x reference)
"""Trainium2 Bass kernel for autoregressive multi-head self-attention.

Problem: B=2, S=2048, H=2048 (16 heads x 128), RoPE, causal softmax with the
(faithful-to-source) sqrt(head_dim) score MULTIPLIER, out projection.

Sharding: 8 cores = 2 (batch) x 4 (head-groups of 4 heads). Attention is fully
local per core. Out-proj is row-parallel: each core emits a partial [H, S]
(transposed) output; host transposes + sums the 4 partials per batch element.

Matmuls run as float32r (fp32 storage, 1 cycle/row at N>=256); PV and the
out-projection run bf16. Softmax is flash-style (per-chunk max + folded
correction) in exact fp32 on DVE/ACT. PV is software-pipelined one supertile
behind the projection; out-proj s-blocks 0-2 are emitted inside pass 1 right
after their supertile's PV so only s-block 3 remains as a serial tail.
Output partials are written bf16 (summed in fp32 on host) to cut HBM traffic.
"""

import math
import sys

sys.path.insert(0, "/opt/trn_rl_repo")

import ml_dtypes
import numpy as np

import concourse.bacc as bacc
import concourse.tile as tile
from concourse import bass_utils, mybir
from contextlib import ExitStack

P = 128          # partitions / head dim / q,k,v tile
S = 2048         # sequence length
H = 2048         # hidden
NH = 16          # total heads
HPC = 4          # heads per core
NCORES = 8
SC = 512         # s-chunk width for projections
NCT = H // P     # 16 c-tiles (contraction)
NQT = S // P     # 16 q tiles
NEG = -1.0e30

R32 = mybir.dt.float32r
F32 = mybir.dt.float32
BF16 = mybir.dt.bfloat16
AX = mybir.AxisListType.X
EXP = mybir.ActivationFunctionType.Exp
ACOPY = mybir.ActivationFunctionType.Copy

SQHD = math.sqrt(P)  # the faithful-to-source sqrt(head_dim) score multiplier

CFG = {"pqk": 1, "psc": 5, "pcx": 1, "xb": 4, "probs": 2, "pta": 2,
       "qt": 2, "m": 1, "trig": 2, "sch": 512}


def _build_program(loop_iters=None):
    nc = bacc.Bacc("TRN2", target_bir_lowering=False, debug=False)

    xT = nc.dram_tensor("xT", [H, S], R32, kind="ExternalInput")        # x[b].T
    wqT = nc.dram_tensor("wqT", [H, HPC * P], R32, kind="ExternalInput")
    wkT = nc.dram_tensor("wkT", [H, HPC * P], R32, kind="ExternalInput")
    wvT = nc.dram_tensor("wvT", [H, HPC * P], R32, kind="ExternalInput")
    woT = nc.dram_tensor("woT", [HPC * P, H], BF16, kind="ExternalInput")
    cosd = nc.dram_tensor("cosd", [P, S], R32, kind="ExternalInput")
    sind = nc.dram_tensor("sind", [P, S], R32, kind="ExternalInput")
    permT = nc.dram_tensor("permT", [P, P], R32, kind="ExternalInput")  # rot-half
    maskc = nc.dram_tensor("maskc", [P, P], BF16, kind="ExternalInput")  # causal add
    ident = nc.dram_tensor("ident", [P, P], BF16, kind="ExternalInput")
    out = nc.dram_tensor("out", [H, S], BF16, kind="ExternalOutput")    # partial, T

    CH = CFG["sch"]

    with tile.TileContext(nc) as tc, ExitStack() as ctx:
        if loop_iters is not None:
            ctx.enter_context(tc.For_i(0, loop_iters, 1))
        cpool = ctx.enter_context(tc.tile_pool(name="consts", bufs=1))
        mask_sb = cpool.tile([P, P], BF16, tag="mask", name="mask_sb")
        perm_sb = cpool.tile([P, P], R32, tag="perm", name="perm_sb")
        id_sb = cpool.tile([P, P], BF16, tag="ident", name="id_sb")
        nc.gpsimd.dma_start(out=mask_sb, in_=maskc.ap())
        nc.gpsimd.dma_start(out=perm_sb, in_=permT.ap())
        nc.gpsimd.dma_start(out=id_sb, in_=ident.ap())

        # out-proj weights + staging, resident so s-blocks 0-1 of the
        # projection can run inside pass 1 (shrinks the serial tail)
        wopool = ctx.enter_context(tc.tile_pool(name="wo", bufs=1))
        wo_sb = wopool.tile([P, HPC, H], BF16, tag="wo", name="wo_sb")
        ostpool = ctx.enter_context(tc.tile_pool(name="ost", bufs=4))

        # ctxT[h]: [d=128, S] bf16 per head, alive until the out-projection
        ctxpool = ctx.enter_context(tc.tile_pool(name="ctxp", bufs=1))
        ctxT = [
            ctxpool.tile([P, S], BF16, tag=f"ctxT{h}", name=f"ctxT{h}")
            for h in range(HPC)
        ]

        for hp in range(2):  # head-pair passes: heads {2hp, 2hp+1}
            with ExitStack() as pctx:
                wpool = pctx.enter_context(tc.tile_pool(name=f"w{hp}", bufs=1))
                wq_sb = wpool.tile([P, NCT, 2 * P], R32, tag="wq", name=f"wq{hp}")
                wk_sb = wpool.tile([P, NCT, 2 * P], R32, tag="wk", name=f"wk{hp}")
                wv_sb = wpool.tile([P, NCT, 2 * P], R32, tag="wv", name=f"wv{hp}")
                osl = slice(hp * 2 * P, (hp + 1) * 2 * P)

                def wchunk(wsb, wdr, wg):
                    nc.sync.dma_start(
                        out=wsb[:, wg * 2 : (wg + 1) * 2, :],
                        in_=wdr.ap()[wg * 2 * P : (wg + 1) * 2 * P, osl]
                        .rearrange("(t p) s -> p t s", p=P),
                    )

                kvpool = pctx.enter_context(tc.tile_pool(name=f"kv{hp}", bufs=1))
                kT = [
                    kvpool.tile([P, S], R32, tag=f"kT{i}", name=f"kT{hp}_{i}")
                    for i in range(2)
                ]
                v_sb = kvpool.tile([P, NQT, 2 * P], BF16, tag="v", name=f"v{hp}")

                # merged projection + attention pools
                xpool = pctx.enter_context(tc.tile_pool(name=f"x{hp}", bufs=CFG["xb"]))
                tpool = pctx.enter_context(tc.tile_pool(name=f"t{hp}", bufs=CFG["trig"]))
                mpool = pctx.enter_context(tc.tile_pool(name=f"m{hp}", bufs=CFG["m"]))
                qtpool = pctx.enter_context(tc.tile_pool(name=f"qt{hp}", bufs=CFG["qt"]))
                ppool = pctx.enter_context(tc.tile_pool(name=f"pr{hp}", bufs=CFG["probs"]))
                ptapool = pctx.enter_context(tc.tile_pool(name=f"pt{hp}", bufs=CFG["pta"]))
                smpool = pctx.enter_context(tc.tile_pool(name=f"sm{hp}", bufs=4))
                psqk_pool = pctx.enter_context(
                    tc.tile_pool(name=f"pqk{hp}", bufs=CFG["pqk"], space="PSUM")
                )
                psc_pool = pctx.enter_context(
                    tc.tile_pool(
                        name=f"psc{hp}",
                        bufs=CFG["psc"] if hp == 0 else CFG["psc"] - 2,
                        space="PSUM",
                    )
                )
                if hp == 1:
                    po_pool = pctx.enter_context(
                        tc.tile_pool(name="po", bufs=2, space="PSUM")
                    )
                pcx_pool = pctx.enter_context(
                    tc.tile_pool(name=f"pcx{hp}", bufs=CFG["pcx"], space="PSUM")
                )

                def load_x(sc, g):
                    t = xpool.tile(
                        [P, 4, SC], R32, tag="xb", name=f"xb{hp}_{sc}_{g}"
                    )
                    nc.sync.dma_start(
                        out=t,
                        in_=xT.ap()[g * 4 * P : (g + 1) * 4 * P,
                                    sc * SC : (sc + 1) * SC]
                        .rearrange("(t p) s -> p t s", p=P),
                    )
                    return t

                def load_trig(sc):
                    ssl = slice(sc * SC, (sc + 1) * SC)
                    cqk = tpool.tile([P, SC], R32, tag="ctrig", name=f"cq{hp}_{sc}")
                    sqk = tpool.tile([P, SC], R32, tag="strig", name=f"sq{hp}_{sc}")
                    nc.sync.dma_start(out=cqk, in_=cosd.ap()[:, ssl])
                    nc.sync.dma_start(out=sqk, in_=sind.ap()[:, ssl])
                    return cqk, sqk

                def emit_outproj(sblk, pool):
                    ssl2 = slice(sblk * 512, (sblk + 1) * 512)
                    for oc in range(NCT):
                        po = pool.tile(
                            [P, 512], F32, tag="po", name=f"po{sblk}_{oc}"
                        )
                        for h2 in range(HPC):
                            nc.tensor.matmul(
                                po,
                                lhsT=wo_sb[:, h2, oc * P : (oc + 1) * P],
                                rhs=ctxT[h2][:, ssl2],
                                start=(h2 == 0),
                                stop=(h2 == HPC - 1),
                            )
                        ost = ostpool.tile(
                            [P, 512], BF16, tag="ost", name=f"ost{sblk}_{oc}"
                        )
                        nc.vector.tensor_copy(out=ost, in_=po)
                        nc.sync.dma_start(
                            out=out.ap()[oc * P : (oc + 1) * P, ssl2], in_=ost
                        )

                def emit_pv(Q, ptbs):
                    # PV: left chain (qcols 0-1) over kt <= 4Q+1 needs only
                    # the first two transposes; right chain (2-3) all four.
                    for hh in range(2):
                        h = hp * 2 + hh
                        ptb_l, ptb_r = ptbs[hh]
                        ctps = pcx_pool.tile(
                            [P, 512], F32, tag="cx", name=f"cx{h}{Q}"
                        )
                        for reg, ptbh, nkt in (
                            (slice(0, 256), ptb_l, Q * 4 + 2),
                            (slice(256, 512), ptb_r, Q * 4 + 4),
                        ):
                            for kt in range(nkt):
                                nc.tensor.matmul(
                                    ctps[:, reg],
                                    lhsT=v_sb[:, kt, hh * P : (hh + 1) * P],
                                    rhs=ptbh[:, kt, :, :],
                                    start=(kt == 0),
                                    stop=(kt == nkt - 1),
                                )
                        nc.scalar.copy(
                            out=ctxT[h][:, Q * 512 : (Q + 1) * 512], in_=ctps
                        )

                pending_pv = None

                # sc=0 loads interleaved with the weight chunks, in the order
                # the PE will consume them (QK weights / x first, V last).
                wchunk(wq_sb, wqT, 0)
                wchunk(wk_sb, wkT, 0)
                xg0 = [load_x(0, 0), load_x(0, 1)]
                for wg in range(1, 8):
                    wchunk(wq_sb, wqT, wg)
                    wchunk(wk_sb, wkT, wg)
                for wg in range(8):
                    wchunk(wv_sb, wvT, wg)
                xg0 += [load_x(0, 2), load_x(0, 3)]
                trig0 = load_trig(0)
                if hp == 0:  # prefetch out-proj weights behind pass-0 loads
                    nc.sync.dma_start(
                        out=wo_sb,
                        in_=woT.ap().rearrange("(t p) o -> p t o", p=P),
                    )

                for sc in range(S // SC):
                    ssl = slice(sc * SC, (sc + 1) * SC)
                    if sc == 0:
                        xg, (cqk, sqk) = xg0, trig0
                    else:
                        xg = [load_x(sc, g) for g in range(4)]
                        cqk, sqk = load_trig(sc)
                    xb = [xg[ct // 4][:, ct % 4, :] for ct in range(NCT)]

                    qts = []
                    for hh in range(2):
                        hsl = slice(hh * P, (hh + 1) * P)
                        psqk = psqk_pool.tile(
                            [P, 2 * SC], F32, tag="pqk", name=f"pqk{hp}{sc}{hh}"
                        )
                        pq = psqk[:, :SC]
                        pk = psqk[:, SC:]
                        for ct in range(NCT):
                            nc.tensor.matmul(
                                pq,
                                lhsT=wq_sb[:, ct, hsl],
                                rhs=xb[ct],
                                start=(ct == 0),
                                stop=(ct == NCT - 1),
                            )
                            nc.tensor.matmul(
                                pk,
                                lhsT=wk_sb[:, ct, hsl],
                                rhs=xb[ct],
                                start=(ct == 0),
                                stop=(ct == NCT - 1),
                            )
                        # ---- RoPE: q into qt, k into kT[hh][:, ssl] ----
                        raw = mpool.tile(
                            [P, 2 * SC], R32, tag="raw", name=f"raw{hp}{sc}{hh}"
                        )
                        nc.scalar.copy(out=raw, in_=psqk)
                        qt = qtpool.tile(
                            [P, SC], R32, tag=f"qt{hh}", name=f"qt{hp}{sc}{hh}"
                        )
                        qts.append(qt)
                        for half, dest in ((0, qt), (1, kT[hh][:, ssl])):
                            hsl2 = slice(half * SC, (half + 1) * SC)
                            rot = psc_pool.tile(
                                [P, SC], F32, tag="sc",
                                name=f"rot{hp}{sc}{hh}{half}"
                            )
                            nc.tensor.matmul(
                                rot, lhsT=perm_sb, rhs=raw[:, hsl2],
                                start=True, stop=True,
                            )
                            nc.gpsimd.tensor_mul(
                                out=dest, in0=raw[:, hsl2], in1=cqk
                            )
                            tmp = mpool.tile(
                                [P, SC], F32, tag="rtmp",
                                name=f"tmp{hp}{sc}{hh}{half}"
                            )
                            nc.vector.tensor_mul(out=tmp, in0=rot, in1=sqk)
                            nc.gpsimd.tensor_add(out=dest, in0=dest, in1=tmp)

                    for sti in range(SC // P):
                        st = sc * (SC // P) + sti
                        psv = psc_pool.tile(
                            [P, SC], F32, tag="sc", name=f"pv{hp}{st}"
                        )
                        for ct in range(NCT):
                            nc.tensor.matmul(
                                psv[:, : 2 * P],
                                lhsT=xb[ct][:, sti * P : (sti + 1) * P],
                                rhs=wv_sb[:, ct, :],
                                start=(ct == 0),
                                stop=(ct == NCT - 1),
                            )
                        nc.vector.tensor_copy(
                            out=v_sb[:, st, :], in_=psv[:, : 2 * P]
                        )

                    # PV for the previous supertile runs here, after this
                    # chunk's projection: its transposes finished long ago, so
                    # the softmax/transpose latency is fully hidden.
                    if pending_pv is not None:
                        Qp = pending_pv[0]
                        emit_pv(*pending_pv)
                        pending_pv = None
                        if hp == 1 and Qp < 3:
                            emit_outproj(Qp, po_pool)

                    # ---- attention supertile Q = sc for both heads ----
                    Q = sc
                    ptbs = []
                    for hh in range(2):
                        h = hp * 2 + hh
                        # probsT split into qcol halves so the left PV chain
                        # can start after only two transposes: [k, kt, qc, q]
                        ptb_l = ptapool.tile(
                            [P, NQT, 2, P], BF16, tag="ptal", name=f"ptl{h}{Q}"
                        )
                        ptb_r = ptapool.tile(
                            [P, NQT, 2, P], BF16, tag="ptar", name=f"ptr{h}{Q}"
                        )
                        ptbs.append((ptb_l, ptb_r))
                        # zero the causal-overhang blocks (kt > qi)
                        nc.any.memset(ptb_l[:, Q * 4 + 1, 0, :], 0.0)
                        nc.any.memset(ptb_r[:, Q * 4 + 3, 0, :], 0.0)
                        for qi_in in range(4):
                            qi = Q * 4 + qi_in
                            L = (qi + 1) * P
                            nch = (L + CH - 1) // CH
                            pt = ppool.tile(
                                [P, S], BF16, tag="probs", name=f"pr{h}{qi}"
                            )
                            # flash-style: each chunk exps against its OWN max
                            # (frees the psc bank immediately); the correction
                            # exp(m_c - m_row) folds into the normalize scale.
                            maxn = smpool.tile(
                                [P, 4], F32, tag="maxp", name=f"mx{h}{qi}"
                            )
                            sums = smpool.tile(
                                [P, 4], F32, tag="sums", name=f"sm{h}{qi}"
                            )
                            chunks = []
                            for cn in range(nch):
                                n0 = cn * CH
                                w = min(L, n0 + CH) - n0
                                chunks.append((n0, w))
                                psc = psc_pool.tile(
                                    [P, CH], F32, tag="sc", name=f"sc{h}{qi}{cn}"
                                )
                                # matmuls split at PSUM bank boundaries;
                                # diagonal block is always the row tail
                                for j0 in range(0, w, 512):
                                    jw = min(w, j0 + 512) - j0
                                    has_diag = (
                                        n0 + j0 <= qi * P < n0 + j0 + jw
                                    )
                                    nc.tensor.matmul(
                                        psc[:, j0 : j0 + jw],
                                        lhsT=qts[hh][
                                            :, qi_in * P : (qi_in + 1) * P
                                        ],
                                        rhs=kT[hh][:, n0 + j0 : n0 + j0 + jw],
                                        start=True,
                                        stop=not has_diag,
                                    )
                                    if has_diag:
                                        # causal mask as PE accum (bf16 rhs)
                                        off = qi * P - n0
                                        nc.tensor.matmul(
                                            psc[:, off : off + P],
                                            lhsT=id_sb,
                                            rhs=mask_sb,
                                            start=False,
                                            stop=True,
                                        )
                                nc.vector.reduce_max(
                                    out=maxn[:, cn : cn + 1],
                                    in_=psc[:, :w],
                                    axis=AX,
                                    negate=True,
                                )
                                nc.scalar.activation(
                                    out=pt[:, n0 : n0 + w],
                                    in_=psc[:, :w],
                                    func=EXP,
                                    bias=maxn[:, cn : cn + 1],
                                    scale=1.0,
                                    accum_out=sums[:, cn : cn + 1],
                                )
                            recip = smpool.tile(
                                [P, 1], F32, tag="recip", name=f"rc{h}{qi}"
                            )
                            if nch == 1:
                                nc.vector.reciprocal(out=recip, in_=sums[:, 0:1])
                                nc.vector.tensor_scalar_mul(
                                    pt[:, :L], pt[:, :L], recip
                                )
                            else:
                                # maxn holds -m_c; f_c = exp(m_c - m_row)
                                rmin = smpool.tile(
                                    [P, 1], F32, tag="rneg", name=f"rn{h}{qi}"
                                )
                                nc.vector.reduce_max(
                                    out=rmin, in_=maxn[:, :nch], axis=AX,
                                    op=mybir.AluOpType.min,
                                )
                                fac = smpool.tile(
                                    [P, 4], F32, tag="fac", name=f"fc{h}{qi}"
                                )
                                nc.scalar.activation(
                                    out=fac[:, :nch],
                                    in_=maxn[:, :nch],
                                    func=EXP,
                                    bias=rmin,
                                    scale=-1.0,
                                )
                                nc.vector.tensor_mul(
                                    out=sums[:, :nch], in0=sums[:, :nch],
                                    in1=fac[:, :nch],
                                )
                                ssum = smpool.tile(
                                    [P, 1], F32, tag="ssum", name=f"ss{h}{qi}"
                                )
                                nc.vector.reduce_sum(
                                    out=ssum, in_=sums[:, :nch], axis=AX
                                )
                                nc.vector.reciprocal(out=recip, in_=ssum)
                                nc.vector.tensor_scalar_mul(
                                    fac[:, :nch], fac[:, :nch], recip
                                )
                                for cn, (n0, w) in enumerate(chunks):
                                    nc.vector.tensor_scalar_mul(
                                        pt[:, n0 : n0 + w], pt[:, n0 : n0 + w],
                                        fac[:, cn : cn + 1],
                                    )

                            # one batched xbar transpose: [q, L] -> [k, kt, q]
                            half = ptb_l if qi_in < 2 else ptb_r
                            nc.scalar.dma_start(
                                out=half[:, : qi + 1, qi_in % 2, :],
                                in_=pt[:, :L],
                                transpose=True,
                            )

                    pending_pv = (Q, ptbs)

                # flush the last supertile's PV before the pass scope closes
                if pending_pv is not None:
                    emit_pv(*pending_pv)
                    pending_pv = None

        # ---- Phase D tail: remaining s-blocks of the out projection
        with ExitStack() as dctx:
            pod_pool = dctx.enter_context(
                tc.tile_pool(name="pod", bufs=8, space="PSUM")
            )
            for sblk in (3,):
                ssl2 = slice(sblk * 512, (sblk + 1) * 512)
                for oc in range(NCT):
                    po = pod_pool.tile(
                        [P, 512], F32, tag="po", name=f"pod{sblk}_{oc}"
                    )
                    for h2 in range(HPC):
                        nc.tensor.matmul(
                            po,
                            lhsT=wo_sb[:, h2, oc * P : (oc + 1) * P],
                            rhs=ctxT[h2][:, ssl2],
                            start=(h2 == 0),
                            stop=(h2 == HPC - 1),
                        )
                    ost = ostpool.tile(
                        [P, 512], BF16, tag="ost", name=f"osd{sblk}_{oc}"
                    )
                    nc.vector.tensor_copy(out=ost, in_=po)
                    nc.sync.dma_start(
                        out=out.ap()[oc * P : (oc + 1) * P, ssl2], in_=ost
                    )

    nc.compile()
    return nc


_NC_CACHE = None


def _get_program():
    global _NC_CACHE
    if _NC_CACHE is None:
        _NC_CACHE = _build_program()
    return _NC_CACHE


def _host_inputs(x, Wq, Wk, Wv, Wo, cos, sin):
    """Build the 8 per-core input maps (host-side sharding + layout prep)."""
    B = x.shape[0]

    cosT = np.ascontiguousarray(cos[:S].T.astype(np.float32))  # [128, S]
    sinT = np.ascontiguousarray(sin[:S].T.astype(np.float32))

    # rotate-half as a signed permutation: rot[d] = sign(d) * x[(d+64) % 128]
    perm = np.zeros((P, P), np.float32)
    for d in range(P):
        perm[d, (d + P // 2) % P] = -1.0 if d < P // 2 else 1.0
    permT_np = np.ascontiguousarray(perm.T)

    mask_np = np.triu(np.full((P, P), NEG, np.float32), k=1).astype(
        ml_dtypes.bfloat16
    )
    ident_np = np.eye(P, dtype=np.float32).astype(ml_dtypes.bfloat16)

    xTb = [np.ascontiguousarray(x[b].T.astype(np.float32)) for b in range(B)]

    in_maps = []
    for core in range(NCORES):
        b = core // 4
        hg = core % 4
        rows = slice(hg * HPC * P, (hg + 1) * HPC * P)
        in_maps.append(
            {
                "xT": xTb[b],
                # sqrt(hd) score multiplier folded into Wq (rope is linear)
                "wqT": np.ascontiguousarray(Wq[rows, :].T.astype(np.float32) * SQHD),
                "wkT": np.ascontiguousarray(Wk[rows, :].T.astype(np.float32)),
                "wvT": np.ascontiguousarray(Wv[rows, :].T.astype(np.float32)),
                "woT": np.ascontiguousarray(Wo[:, rows].T.astype(ml_dtypes.bfloat16)),
                "cosd": cosT,
                "sind": sinT,
                "permT": permT_np,
                "maskc": mask_np,
                "ident": ident_np,
            }
        )
    return in_maps


def kernel(x, Wq, Wk, Wv, Wo, cos, sin, _trace=False):
    x, Wq, Wk, Wv, Wo, cos, sin = (
        np.asarray(a, dtype=np.float32) for a in (x, Wq, Wk, Wv, Wo, cos, sin)
    )
    nc = _get_program()
    in_maps = _host_inputs(x, Wq, Wk, Wv, Wo, cos, sin)
    res = bass_utils.run_bass_kernel_spmd(
        nc, in_maps, core_ids=list(range(NCORES)), trace=_trace
    )
    kernel.last_result = res
    B = x.shape[0]
    out = np.zeros((B, S, H), np.float32)
    for core in range(NCORES):
        out[core // 4] += res.results[core]["out"].T.astype(np.float32)
    return out
